# revision 9
# baseline (speedup 1.0000x reference)
"""AttentionPooling Trainium2 kernel: 8-core data-parallel over batch.

Math notes (validated in check_math.py):
 - Cross-attention softmaxes over a single key position -> identity, so the
   whole first MHA collapses to  c = x @ Wc.T,  Wc = ca_w_out @ ca_wv,
   h1[b,l] = c[b] + (latents[l] + bc).
 - Self-attention over L=4 latents, H=8 heads, dh=48; 1/sqrt(dh) folded
   into Wq.  Softmax computed without max-subtraction (scores are tiny).
 - LN3 affine (g3, b3) folded after the mean-pool (pool is linear).

On-chip layout: rows = (b, l) pairs on partitions (l fastest), 128 rows per
subtile (32 batch); 4 subtiles per 128-batch macro tile; 32 macros per core.
GEMMs run on PE with activations transposed on-chip (PE transpose); norms and
softmax use per-partition fused DVE ops; transcendentals on ACT.
"""

from contextlib import ExitStack

import numpy as np
import ml_dtypes

import concourse.bass as bass
import concourse.bacc as bacc_mod
import concourse.tile as tile
from concourse import mybir
from concourse.bass_utils import run_bass_kernel_spmd

D, H, L, B, NCORES = 384, 8, 4, 32768, 8
DH = D // H                      # 48
BC = B // NCORES                 # 4096 rows per core
P = 128
NMAC = BC // P                   # 32 macro tiles per core
NSUB = 4                         # subtiles per macro (128 (b,l) rows each)
EPS = 1e-5

BF16 = ml_dtypes.bfloat16
f32 = mybir.dt.float32
bf16 = mybir.dt.bfloat16
AL = mybir.AluOpType
AF = mybir.ActivationFunctionType
AX = mybir.AxisListType


def _host_consts(inp):
    """All small parameters fused/packed on the host (numpy, f32 -> bf16)."""
    wq, wk, wv = np.split(inp["ca_w_in"], 3, axis=0)
    _, _, bv = np.split(inp["ca_b_in"], 3)
    Wc = inp["ca_w_out"] @ wv                              # [D, D]
    bc = inp["ca_w_out"] @ bv + inp["ca_b_out"]            # [D]
    latb = inp["latents"][0] + bc[None, :]                 # [L, D]

    WsaT = inp["sa_w_in"].T.copy()                         # [D, 3D]
    WsaT[:, :D] *= 1.0 / np.sqrt(DH)
    bqkv = inp["sa_b_in"].copy()
    bqkv[:D] *= 1.0 / np.sqrt(DH)

    def chunkT(wT, nk):  # [D_in, N] -> [128, nk, N] (k-chunks of 128 on partitions)
        n = wT.shape[1]
        return np.ascontiguousarray(wT.reshape(nk, P, n).transpose(1, 0, 2))

    c = {}
    c["wc"] = chunkT(Wc.T, 3)                              # [128, 3, 384]
    c["wsa"] = chunkT(WsaT, 3)                             # [128, 3, 1152]
    c["wso"] = chunkT(inp["sa_w_out"].T, 3)                # [128, 3, 384]
    c["w1"] = chunkT(inp["ffn_w1"].T, 3)                   # [128, 3, 1536]
    c["w2"] = chunkT(inp["ffn_w2"].T, 12)                  # [128, 12, 384]
    c["wg"] = chunkT(inp["gate_w"].T, 3)                   # [128, 3, 384]
    c["latb"] = latb                                       # [4, 384]

    # expansion / broadcast / pooling matrices
    pidx = np.arange(P)
    Eall = np.zeros((P, NSUB, P), np.float32)              # lhsT: [b128, s, p]
    for s in range(NSUB):
        Eall[32 * s + pidx // L, s, pidx] = 1.0
    c["emat"] = Eall
    oneL = np.zeros((L, P), np.float32)
    oneL[pidx % L, pidx] = 1.0
    c["onel"] = oneL                                       # [4, 128]
    Bl = np.zeros((P, L, P), np.float32)                   # lhsT: [p', l', p]
    for lp in range(L):
        Bl[(pidx // L) * L + lp, lp, pidx] = 1.0
    c["bl"] = Bl
    pm = np.zeros((P, 32), np.float32)
    pm[pidx, pidx // L] = 0.25
    c["pool"] = pm                                         # [128, 32]
    c["ident"] = np.eye(P, dtype=np.float32)

    # bias rows for PE bias-add matmuls (rank-1 tricks)
    c["ones1"] = np.ones((1, P), np.float32)
    c["bqkv"] = bqkv[None, :]                              # [1, 1152]
    c["bso"] = inp["sa_b_out"][None, :]
    c["b1row"] = inp["ffn_b1"][None, :]                    # [1, 1536]
    c["b2row"] = inp["ffn_b2"][None, :]
    c["bgrow"] = inp["gate_b"][None, :]

    # replicated per-feature vectors [128, 384]
    for nm in ("n1_g", "n1_b", "n2_g", "n2_b", "n3_g", "n3_b"):
        c[nm] = np.broadcast_to(inp[nm][None, :], (P, D)).copy()
    return {k: v.astype(BF16) for k, v in c.items()}


def _fancy(apbase, free_dims, extra_elem_offset=0):
    """Build an AP with custom free dims [[step,count],...] on top of a tile AP."""
    return bass.AP(
        tensor=apbase.tensor,
        offset=apbase.offset + extra_elem_offset,
        ap=[apbase.ap[0]] + [list(d) for d in free_dims],
    )


def _layernorm_stats(nc, pools, src_ap, nparts=P):
    """-> (rstd[128,1] f32, nmr[128,1] f32) for per-partition LN over free dim."""
    st = pools["stat"].tile([nparts, 6], f32, tag="bnst")
    nc.vector.bn_stats(out=st[:, :], in_=src_ap)
    mv = pools["stat"].tile([nparts, 2], f32, tag="bnmv")
    nc.vector.bn_aggr(out=mv[:, :], in_=st[:, :])
    rstd = pools["stat"].tile([nparts, 1], f32, tag="rstd")
    nc.scalar.activation(out=rstd[:, :], in_=mv[:, 1:2], func=AF.Sqrt,
                         bias=pools["eps"][:nparts, :], scale=1.0)
    nc.vector.reciprocal(out=rstd[:, :], in_=rstd[:, :])
    nmr = pools["stat"].tile([nparts, 1], f32, tag="nmr")
    nc.vector.tensor_scalar(out=nmr[:, :], in0=mv[:, 0:1], scalar1=rstd[:, 0:1],
                            scalar2=-1.0, op0=AL.mult, op1=AL.mult)
    return rstd, nmr


def build_program():
    nc = bacc_mod.Bacc("TRN2", target_bir_lowering=False, debug=False,
                       num_devices=NCORES)
    x_d = nc.declare_dram_parameter("x", [BC, D], f32, isOutput=False)
    consts_meta = {
        "wc": [P, 3, D], "wsa": [P, 3, 3 * D], "wso": [P, 3, D],
        "w1": [P, 3, 4 * D], "w2": [P, 12, D], "wg": [P, 3, D],
        "latb": [L, D], "emat": [P, NSUB, P], "onel": [L, P],
        "bl": [P, L, P], "pool": [P, 32], "ident": [P, P],
        "ones1": [1, P], "bqkv": [1, 3 * D], "bso": [1, D],
        "b1row": [1, 4 * D], "b2row": [1, D], "bgrow": [1, D],
        "n1_g": [P, D], "n1_b": [P, D], "n2_g": [P, D], "n2_b": [P, D],
        "n3_g": [P, D], "n3_b": [P, D],
    }
    cd = {k: nc.declare_dram_parameter(k, shp, bf16, isOutput=False)
          for k, shp in consts_meta.items()}
    out_d = nc.declare_dram_parameter("out", [BC, D], f32, isOutput=True)

    with tile.TileContext(nc) as tc, ExitStack() as ctx:
        consts = ctx.enter_context(tc.tile_pool(name="consts", bufs=1))
        io = ctx.enter_context(tc.tile_pool(name="io", bufs=3))
        act = ctx.enter_context(tc.tile_pool(name="act", bufs=2))
        stat = ctx.enter_context(tc.tile_pool(name="stat", bufs=4))
        ps_med = ctx.enter_context(tc.tile_pool(name="ps_med", bufs=2, space="PSUM"))
        ps_tp = ctx.enter_context(tc.tile_pool(name="ps_tp", bufs=1, space="PSUM"))
        ps_big = ctx.enter_context(tc.tile_pool(name="ps_big", bufs=1, space="PSUM"))
        ps_pool = ctx.enter_context(tc.tile_pool(name="ps_pool", bufs=1, space="PSUM"))

        cs = {}
        for k, shp in consts_meta.items():
            cs[k] = consts.tile(shp, bf16, name=f"c_{k}", tag=f"c_{k}")
            nc.sync.dma_start(out=cs[k][:], in_=cd[k][:])
        eps_t = consts.tile([P, 1], f32, tag="eps")
        nc.vector.memset(eps_t[:, :], EPS)
        pools = {"stat": stat, "eps": eps_t}

        identf = consts.tile([P, P], f32, tag="identf")
        # f32 identity built on-chip from the bf16 one (copy casts)
        nc.vector.tensor_copy(out=identf[:], in_=cs["ident"][:])

        def transpose3(src_ap_fn, nchunks, dst_tag, src_f32=False):
            """PE-transpose nchunks [128,128] blocks of a row-major tile -> bf16 [128,nchunks,128]."""
            dst = act.tile([P, nchunks, P], bf16, tag=dst_tag)
            for j in range(nchunks):
                tp = ps_tp.tile([P, P], f32 if src_f32 else bf16, tag="tp")
                nc.tensor.transpose(tp[:, :], src_ap_fn(j),
                                    identf[:] if src_f32 else cs["ident"][:])
                nc.any.tensor_copy(out=dst[:, j, :], in_=tp[:, :])
            return dst

        for m in range(NMAC):
            xt = io.tile([P, D], f32, tag="xin")
            nc.sync.dma_start(out=xt[:], in_=x_d[m * P:(m + 1) * P, :])
            xT = transpose3(lambda j: xt[:, j * P:(j + 1) * P], 3, "xT", src_f32=True)

            # c = x @ Wc.T  (batch-major out [128b, 384])
            cps = ps_med.tile([P, D], f32, tag="med")
            for k in range(3):
                nc.tensor.matmul(cps[:, :], xT[:, k, :], cs["wc"][:, k, :],
                                 start=(k == 0), stop=(k == 2))
            c_sb = io.tile([P, D], bf16, tag="c_sb")
            nc.any.tensor_copy(out=c_sb[:], in_=cps[:, :])

            poolps = ps_pool.tile([P, D], f32, tag="poolacc")

            for s in range(NSUB):
                # ---- h1 = expand(c) + latb ; LN1 ----
                h1ps = ps_med.tile([P, D], f32, tag="med")
                nc.tensor.matmul(h1ps[:, :], cs["emat"][:, s, :], c_sb[:],
                                 start=True, stop=False)
                nc.tensor.matmul(h1ps[:, :], cs["onel"][:, :], cs["latb"][:, :],
                                 start=False, stop=True)
                rstd, nmr = _layernorm_stats(nc, pools, h1ps[:, :])
                y0 = act.tile([P, D], bf16, tag="y0")
                nc.vector.tensor_scalar(out=y0[:], in0=h1ps[:, :],
                                        scalar1=rstd[:, 0:1], scalar2=nmr[:, 0:1],
                                        op0=AL.mult, op1=AL.add)
                y1 = act.tile([P, D], bf16, tag="y1")
                nc.vector.tensor_mul(y1[:], y0[:], cs["n1_g"][:])
                nc.vector.tensor_add(y1[:], y1[:], cs["n1_b"][:])

                # ---- qkv GEMM ----
                y1T = transpose3(lambda j: y1[:, j * P:(j + 1) * P], 3, "y1T")
                qkvps = ps_big.tile([P, 4, 512], f32, tag="big")
                for part in range(3):          # q, k, v
                    for k in range(3):
                        nc.tensor.matmul(qkvps[:, part, 0:D], y1T[:, k, :],
                                         cs["wsa"][:, k, part * D:(part + 1) * D],
                                         start=(k == 0), stop=False)
                    nc.tensor.matmul(qkvps[:, part, 0:D], cs["ones1"][:, :],
                                     cs["bqkv"][:, part * D:(part + 1) * D],
                                     start=False, stop=True)
                qkv = act.tile([P, 3, D], bf16, tag="qkv")
                for part in range(3):
                    nc.any.tensor_copy(out=qkv[:, part, :], in_=qkvps[:, part, 0:D])

                # ---- scores: s[p, l', h] = sum_d q[p,h,d] * k[(b,l'),h,d] ----
                kx = ps_big.tile([P, L, 512], f32, tag="big")
                for lp in range(L):
                    nc.tensor.matmul(kx[:, lp, 0:D], cs["bl"][:, lp, :],
                                     qkv[:, 1, :], start=True, stop=True)
                t1 = act.tile([P, L, H, DH], bf16, tag="tbig")
                q_bcast = _fancy(qkv[:, 0, :], [[0, L], [DH, H], [1, DH]])
                kx_view = _fancy(kx[:, 0, 0:D], [[512, L], [DH, H], [1, DH]])
                nc.vector.tensor_mul(t1[:], q_bcast, kx_view)
                s_f = act.tile([P, L, H], f32, tag="s_f")
                nc.vector.reduce_sum(out=s_f[:], in_=t1[:], axis=AX.X)

                # softmax over l' (no max-subtract; scores are tiny)
                e_t = act.tile([P, L, H], f32, tag="e_t")
                nc.scalar.activation(out=e_t[:], in_=s_f[:], func=AF.Exp)
                z_t = act.tile([P, H], f32, tag="z_t")
                nc.vector.reduce_sum(out=z_t[:],
                                     in_=_fancy(e_t[:, 0, :], [[1, H], [H, L]]),
                                     axis=AX.X)
                nc.vector.reciprocal(out=z_t[:], in_=z_t[:])
                a_t = act.tile([P, L, H], f32, tag="a_t")
                nc.vector.tensor_mul(a_t[:], e_t[:],
                                     _fancy(z_t[:, :], [[0, L], [1, H]]))

                # ---- o = sum_l' a * v ----
                vx = ps_big.tile([P, L, 512], f32, tag="big")
                for lp in range(L):
                    nc.tensor.matmul(vx[:, lp, 0:D], cs["bl"][:, lp, :],
                                     qkv[:, 2, :], start=True, stop=True)
                t2 = act.tile([P, L, H, DH], bf16, tag="tbig")
                a_bcast = _fancy(a_t[:, 0, 0:1], [[H, L], [1, H], [0, DH]])
                vx_view = _fancy(vx[:, 0, 0:D], [[512, L], [DH, H], [1, DH]])
                nc.vector.tensor_mul(t2[:], a_bcast, vx_view)
                o_sb = act.tile([P, D], f32, tag="o_sb")
                nc.vector.reduce_sum(
                    out=o_sb[:],
                    in_=_fancy(t2[:, 0, 0, :], [[DH, H], [1, DH], [H * DH, L]]),
                    axis=AX.X)

                # ---- out-proj + residual + LN2 ----
                oT = transpose3(lambda j: o_sb[:, j * P:(j + 1) * P], 3, "oT",
                                src_f32=True)
                h2ps = ps_med.tile([P, D], f32, tag="med")
                for k in range(3):
                    nc.tensor.matmul(h2ps[:, :], oT[:, k, :], cs["wso"][:, k, :],
                                     start=(k == 0), stop=False)
                nc.tensor.matmul(h2ps[:, :], cs["ones1"][:, :], cs["bso"][:, :],
                                 start=False, stop=True)
                r2 = act.tile([P, D], bf16, tag="r2")
                nc.vector.tensor_add(r2[:], h2ps[:, :], y1[:])
                rstd, nmr = _layernorm_stats(nc, pools, r2[:])
                y2 = act.tile([P, D], bf16, tag="y2")
                nc.vector.tensor_scalar(out=y2[:], in0=r2[:],
                                        scalar1=rstd[:, 0:1], scalar2=nmr[:, 0:1],
                                        op0=AL.mult, op1=AL.add)
                nc.vector.tensor_mul(y2[:], y2[:], cs["n2_g"][:])
                nc.vector.tensor_add(y2[:], y2[:], cs["n2_b"][:])

                # ---- FFN (hidden stays feature-major: gelu output == lhsT chunks) ----
                y2T = transpose3(lambda j: y2[:, j * P:(j + 1) * P], 3, "y2T")
                ff1 = ps_big.tile([P, 12, P], f32, tag="big")
                for cchunk in range(12):
                    for k in range(3):
                        nc.tensor.matmul(ff1[:, cchunk, :], cs["w1"][:, k, cchunk * P:(cchunk + 1) * P],
                                         y2T[:, k, :], start=(k == 0), stop=False)
                    nc.tensor.matmul(ff1[:, cchunk, :],
                                     cs["b1row"][:, cchunk * P:(cchunk + 1) * P],
                                     cs["ones1"][:, :], start=False, stop=True)
                gl = act.tile([P, 12, P], bf16, tag="gl")
                nc.scalar.activation(out=gl[:], in_=ff1[:], func=AF.Gelu)
                ff2 = ps_med.tile([P, D], f32, tag="med")
                for k in range(12):
                    nc.tensor.matmul(ff2[:, :], gl[:, k, :], cs["w2"][:, k, :],
                                     start=(k == 0), stop=False)
                nc.tensor.matmul(ff2[:, :], cs["ones1"][:, :], cs["b2row"][:, :],
                                 start=False, stop=True)
                r3 = act.tile([P, D], bf16, tag="r3")
                nc.vector.tensor_add(r3[:], ff2[:, :], y2[:])
                rstd, nmr = _layernorm_stats(nc, pools, r3[:])
                y3 = act.tile([P, D], bf16, tag="y3")
                nc.vector.tensor_scalar(out=y3[:], in0=r3[:],
                                        scalar1=rstd[:, 0:1], scalar2=nmr[:, 0:1],
                                        op0=AL.mult, op1=AL.add)

                # ---- pool over l (accumulate [32,384] slices into macro psum) ----
                nc.tensor.matmul(poolps[32 * s:32 * (s + 1), :], cs["pool"][:, :],
                                 y3[:], start=True, stop=True,
                                 tile_position=(0, 32 * s))

            # ---- LN3 affine (folded post-pool) + gate + output ----
            pooled = io.tile([P, D], bf16, tag="pooled")
            nc.vector.tensor_mul(pooled[:], poolps[:, :], cs["n3_g"][:])
            nc.vector.tensor_add(pooled[:], pooled[:], cs["n3_b"][:])
            pT = transpose3(lambda j: pooled[:, j * P:(j + 1) * P], 3, "pT")
            gps = ps_med.tile([P, D], f32, tag="med")
            for k in range(3):
                nc.tensor.matmul(gps[:, :], pT[:, k, :], cs["wg"][:, k, :],
                                 start=(k == 0), stop=False)
            nc.tensor.matmul(gps[:, :], cs["ones1"][:, :], cs["bgrow"][:, :],
                             start=False, stop=True)
            gsig = io.tile([P, D], bf16, tag="gsig")
            nc.scalar.activation(out=gsig[:], in_=gps[:, :], func=AF.Sigmoid)
            outf = io.tile([P, D], f32, tag="outf")
            nc.vector.tensor_mul(outf[:], pooled[:], gsig[:])
            nc.sync.dma_start(out=out_d[m * P:(m + 1) * P, :], in_=outf[:])

    nc.finalize()   # Bacc: full compile pipeline (wait legalization etc.)
    return nc


_prog = None


def kernel(**inputs):
    global _prog
    inputs = {k: np.asarray(v, dtype=np.float32) for k, v in inputs.items()}
    consts = _host_consts(inputs)
    if _prog is None:
        _prog = build_program()
    x = inputs["x"]
    in_maps = []
    for c in range(NCORES):
        m = {"x": np.ascontiguousarray(x[c * BC:(c + 1) * BC])}
        m.update(consts)
        in_maps.append(m)
    res = run_bass_kernel_spmd(_prog, in_maps, core_ids=list(range(NCORES)))
    return np.concatenate([res.results[c]["out"] for c in range(NCORES)], axis=0)


if __name__ == "__main__":
    rng = np.random.default_rng(0)
    fake = {"x": rng.standard_normal((B, D), dtype=np.float32)}
    print("smoke build only")
    build_program()
    print("build OK")


# revision 10
# speedup vs baseline: 219.5000x; 219.5000x over previous
"""AttentionPooling Trainium2 kernel: 8-core data-parallel over batch.

Math notes (validated in check_math.py):
 - Cross-attention softmaxes over a single key position -> identity, so the
   whole first MHA collapses to  c = x @ Wc.T,  Wc = ca_w_out @ ca_wv,
   h1[b,l] = c[b] + (latents[l] + bc).
 - Self-attention over L=4 latents, H=8 heads, dh=48; 1/sqrt(dh) folded
   into Wq.  Softmax computed without max-subtraction (scores are tiny).
 - LN3 affine (g3, b3) folded after the mean-pool (pool is linear).

On-chip layout: rows = (b, l) pairs on partitions (l fastest), 128 rows per
subtile (32 batch); 4 subtiles per 128-batch macro tile; 32 macros per core.
GEMMs run on PE with activations transposed on-chip (PE transpose); norms and
softmax use per-partition fused DVE ops; transcendentals on ACT.
"""

from contextlib import ExitStack

import numpy as np
import ml_dtypes

import concourse.bass as bass
import concourse.bacc as bacc_mod
import concourse.tile as tile
from concourse import mybir
from concourse.bass_utils import run_bass_kernel_spmd

D, H, L, B, NCORES = 384, 8, 4, 32768, 8
DH = D // H                      # 48
BC = B // NCORES                 # 4096 rows per core
P = 128
NMAC = BC // P                   # 32 macro tiles per core
NSUB = 4                         # subtiles per macro (128 (b,l) rows each)
EPS = 1e-5

BF16 = ml_dtypes.bfloat16
f32 = mybir.dt.float32
bf16 = mybir.dt.bfloat16
AL = mybir.AluOpType
AF = mybir.ActivationFunctionType
AX = mybir.AxisListType


def _host_consts(inp):
    """All small parameters fused/packed on the host (numpy, f32 -> bf16)."""
    wq, wk, wv = np.split(inp["ca_w_in"], 3, axis=0)
    _, _, bv = np.split(inp["ca_b_in"], 3)
    Wc = inp["ca_w_out"] @ wv                              # [D, D]
    bc = inp["ca_w_out"] @ bv + inp["ca_b_out"]            # [D]
    latb = inp["latents"][0] + bc[None, :]                 # [L, D]

    WsaT = inp["sa_w_in"].T.copy()                         # [D, 3D]
    WsaT[:, :D] *= 1.0 / np.sqrt(DH)
    bqkv = inp["sa_b_in"].copy()
    bqkv[:D] *= 1.0 / np.sqrt(DH)

    def chunkT(wT, nk):  # [D_in, N] -> [128, nk, N] (k-chunks of 128 on partitions)
        n = wT.shape[1]
        return np.ascontiguousarray(wT.reshape(nk, P, n).transpose(1, 0, 2))

    c = {}
    c["wc"] = chunkT(Wc.T, 3)                              # [128, 3, 384]
    c["wsa"] = chunkT(WsaT, 3)                             # [128, 3, 1152]
    c["wso"] = chunkT(inp["sa_w_out"].T, 3)                # [128, 3, 384]
    c["w1"] = chunkT(inp["ffn_w1"].T, 3)                   # [128, 3, 1536]
    c["w2"] = chunkT(inp["ffn_w2"].T, 12)                  # [128, 12, 384]
    c["wg"] = chunkT(inp["gate_w"].T, 3)                   # [128, 3, 384]
    c["latb"] = latb                                       # [4, 384]

    # expansion / broadcast / pooling matrices
    pidx = np.arange(P)
    Eall = np.zeros((P, NSUB, P), np.float32)              # lhsT: [b128, s, p]
    for s in range(NSUB):
        Eall[32 * s + pidx // L, s, pidx] = 1.0
    c["emat"] = Eall
    oneL = np.zeros((L, P), np.float32)
    oneL[pidx % L, pidx] = 1.0
    c["onel"] = oneL                                       # [4, 128]
    Bl = np.zeros((P, L, P), np.float32)                   # lhsT: [p', l', p]
    for lp in range(L):
        Bl[(pidx // L) * L + lp, lp, pidx] = 1.0
    c["bl"] = Bl
    pm = np.zeros((P, 32), np.float32)
    pm[pidx, pidx // L] = 0.25
    c["pool"] = pm                                         # [128, 32]
    c["ident"] = np.eye(P, dtype=np.float32)

    # bias rows for PE bias-add matmuls (rank-1 tricks)
    c["ones1"] = np.ones((1, P), np.float32)
    c["bqkv"] = bqkv[None, :]                              # [1, 1152]
    c["bso"] = inp["sa_b_out"][None, :]
    c["b1row"] = inp["ffn_b1"][None, :]                    # [1, 1536]
    c["b2row"] = inp["ffn_b2"][None, :]
    c["bgrow"] = inp["gate_b"][None, :]

    # replicated per-feature vectors [128, 384]
    for nm in ("n1_g", "n1_b", "n2_g", "n2_b", "n3_g", "n3_b"):
        c[nm] = np.broadcast_to(inp[nm][None, :], (P, D)).copy()
    return {k: v.astype(BF16) for k, v in c.items()}


def _fancy(apbase, free_dims, extra_elem_offset=0):
    """Build an AP with custom free dims [[step,count],...] on top of a tile AP."""
    return bass.AP(
        tensor=apbase.tensor,
        offset=apbase.offset + extra_elem_offset,
        ap=[apbase.ap[0]] + [list(d) for d in free_dims],
    )


def _layernorm_stats(nc, pools, src_ap, nparts=P):
    """-> (rstd[128,1] f32, nmr[128,1] f32) for per-partition LN over free dim."""
    st = pools["stat"].tile([nparts, 6], f32, tag="bnst")
    nc.vector.bn_stats(out=st[:, :], in_=src_ap)
    mv = pools["stat"].tile([nparts, 2], f32, tag="bnmv")
    nc.vector.bn_aggr(out=mv[:, :], in_=st[:, :])
    rstd = pools["stat"].tile([nparts, 1], f32, tag="rstd")
    nc.scalar.activation(out=rstd[:, :], in_=mv[:, 1:2], func=AF.Sqrt,
                         bias=pools["eps"][:nparts, :], scale=1.0)
    nc.vector.reciprocal(out=rstd[:, :], in_=rstd[:, :])
    nmr = pools["stat"].tile([nparts, 1], f32, tag="nmr")
    nc.vector.tensor_scalar(out=nmr[:, :], in0=mv[:, 0:1], scalar1=rstd[:, 0:1],
                            scalar2=-1.0, op0=AL.mult, op1=AL.mult)
    return rstd, nmr


def build_program(repeat=1):
    nc = bacc_mod.Bacc("TRN2", target_bir_lowering=False, debug=False,
                       num_devices=NCORES)
    x_d = nc.declare_dram_parameter("x", [BC, D], f32, isOutput=False)
    consts_meta = {
        "wc": [P, 3, D], "wsa": [P, 3, 3 * D], "wso": [P, 3, D],
        "w1": [P, 3, 4 * D], "w2": [P, 12, D], "wg": [P, 3, D],
        "latb": [L, D], "emat": [P, NSUB, P], "onel": [L, P],
        "bl": [P, L, P], "pool": [P, 32], "ident": [P, P],
        "ones1": [1, P], "bqkv": [1, 3 * D], "bso": [1, D],
        "b1row": [1, 4 * D], "b2row": [1, D], "bgrow": [1, D],
        "n1_g": [P, D], "n1_b": [P, D], "n2_g": [P, D], "n2_b": [P, D],
        "n3_g": [P, D], "n3_b": [P, D],
    }
    cd = {k: nc.declare_dram_parameter(k, shp, bf16, isOutput=False)
          for k, shp in consts_meta.items()}
    out_d = nc.declare_dram_parameter("out", [BC, D], f32, isOutput=True)

    with tile.TileContext(nc) as tc, ExitStack() as ctx:
        consts = ctx.enter_context(tc.tile_pool(name="consts", bufs=1))
        io = ctx.enter_context(tc.tile_pool(name="io", bufs=3))
        act = ctx.enter_context(tc.tile_pool(name="act", bufs=2))
        stat = ctx.enter_context(tc.tile_pool(name="stat", bufs=4))
        ps_med = ctx.enter_context(tc.tile_pool(name="ps_med", bufs=2, space="PSUM"))
        ps_tp = ctx.enter_context(tc.tile_pool(name="ps_tp", bufs=1, space="PSUM"))
        ps_big = ctx.enter_context(tc.tile_pool(name="ps_big", bufs=1, space="PSUM"))
        ps_pool = ctx.enter_context(tc.tile_pool(name="ps_pool", bufs=1, space="PSUM"))

        cs = {}
        for k, shp in consts_meta.items():
            cs[k] = consts.tile(shp, bf16, name=f"c_{k}", tag=f"c_{k}")
            nc.sync.dma_start(out=cs[k][:], in_=cd[k][:])
        eps_t = consts.tile([P, 1], f32, tag="eps")
        nc.vector.memset(eps_t[:, :], EPS)
        pools = {"stat": stat, "eps": eps_t}

        identf = consts.tile([P, P], f32, tag="identf")
        # f32 identity built on-chip from the bf16 one (copy casts)
        nc.vector.tensor_copy(out=identf[:], in_=cs["ident"][:])

        rep_ctx = tc.For_i(0, repeat, 1) if repeat > 1 else None

        def transpose3(src_ap_fn, nchunks, dst_tag, src_f32=False):
            """PE-transpose nchunks [128,128] blocks of a row-major tile -> bf16 [128,nchunks,128]."""
            dst = act.tile([P, nchunks, P], bf16, tag=dst_tag)
            for j in range(nchunks):
                tp = ps_tp.tile([P, P], f32 if src_f32 else bf16, tag="tp")
                nc.tensor.transpose(tp[:, :], src_ap_fn(j),
                                    identf[:] if src_f32 else cs["ident"][:])
                nc.any.tensor_copy(out=dst[:, j, :], in_=tp[:, :])
            return dst

        if rep_ctx is not None:
            ctx.enter_context(rep_ctx)
        for m in range(NMAC):
            xt = io.tile([P, D], f32, tag="xin")
            nc.sync.dma_start(out=xt[:], in_=x_d[m * P:(m + 1) * P, :])
            xT = transpose3(lambda j: xt[:, j * P:(j + 1) * P], 3, "xT", src_f32=True)

            # c = x @ Wc.T  (batch-major out [128b, 384])
            cps = ps_med.tile([P, D], f32, tag="med")
            for k in range(3):
                nc.tensor.matmul(cps[:, :], xT[:, k, :], cs["wc"][:, k, :],
                                 start=(k == 0), stop=(k == 2))
            c_sb = io.tile([P, D], bf16, tag="c_sb")
            nc.any.tensor_copy(out=c_sb[:], in_=cps[:, :])

            poolps = ps_pool.tile([P, D], f32, tag="poolacc")

            for s in range(NSUB):
                # ---- h1 = expand(c) + latb ; LN1 ----
                h1ps = ps_med.tile([P, D], f32, tag="med")
                nc.tensor.matmul(h1ps[:, :], cs["emat"][:, s, :], c_sb[:],
                                 start=True, stop=False)
                nc.tensor.matmul(h1ps[:, :], cs["onel"][:, :], cs["latb"][:, :],
                                 start=False, stop=True)
                rstd, nmr = _layernorm_stats(nc, pools, h1ps[:, :])
                y0 = act.tile([P, D], bf16, tag="y0")
                nc.vector.tensor_scalar(out=y0[:], in0=h1ps[:, :],
                                        scalar1=rstd[:, 0:1], scalar2=nmr[:, 0:1],
                                        op0=AL.mult, op1=AL.add)
                y1 = act.tile([P, D], bf16, tag="y1")
                nc.vector.tensor_mul(y1[:], y0[:], cs["n1_g"][:])
                nc.vector.tensor_add(y1[:], y1[:], cs["n1_b"][:])

                # ---- qkv GEMM ----
                y1T = transpose3(lambda j: y1[:, j * P:(j + 1) * P], 3, "y1T")
                qkvps = ps_big.tile([P, 4, 512], f32, tag="big")
                for part in range(3):          # q, k, v
                    for k in range(3):
                        nc.tensor.matmul(qkvps[:, part, 0:D], y1T[:, k, :],
                                         cs["wsa"][:, k, part * D:(part + 1) * D],
                                         start=(k == 0), stop=False)
                    nc.tensor.matmul(qkvps[:, part, 0:D], cs["ones1"][:, :],
                                     cs["bqkv"][:, part * D:(part + 1) * D],
                                     start=False, stop=True)
                qkv = act.tile([P, 3, D], bf16, tag="qkv")
                for part in range(3):
                    nc.any.tensor_copy(out=qkv[:, part, :], in_=qkvps[:, part, 0:D])

                # ---- scores: s[p, l', h] = sum_d q[p,h,d] * k[(b,l'),h,d] ----
                kx = ps_big.tile([P, L, 512], f32, tag="big")
                for lp in range(L):
                    nc.tensor.matmul(kx[:, lp, 0:D], cs["bl"][:, lp, :],
                                     qkv[:, 1, :], start=True, stop=True)
                t1 = act.tile([P, L, H, DH], bf16, tag="tbig")
                q_bcast = _fancy(qkv[:, 0, :], [[0, L], [DH, H], [1, DH]])
                kx_view = _fancy(kx[:, 0, 0:D], [[512, L], [DH, H], [1, DH]])
                nc.vector.tensor_mul(t1[:], q_bcast, kx_view)
                s_f = act.tile([P, L, H], f32, tag="s_f")
                nc.vector.reduce_sum(out=s_f[:], in_=t1[:], axis=AX.X)

                # softmax over l' (no max-subtract; scores are tiny)
                e_t = act.tile([P, L, H], f32, tag="e_t")
                nc.scalar.activation(out=e_t[:], in_=s_f[:], func=AF.Exp)
                z_t = act.tile([P, H], f32, tag="z_t")
                nc.vector.reduce_sum(out=z_t[:],
                                     in_=_fancy(e_t[:, 0, :], [[1, H], [H, L]]),
                                     axis=AX.X)
                nc.vector.reciprocal(out=z_t[:], in_=z_t[:])
                a_t = act.tile([P, L, H], f32, tag="a_t")
                nc.vector.tensor_mul(a_t[:], e_t[:],
                                     _fancy(z_t[:, :], [[0, L], [1, H]]))

                # ---- o = sum_l' a * v ----
                vx = ps_big.tile([P, L, 512], f32, tag="big")
                for lp in range(L):
                    nc.tensor.matmul(vx[:, lp, 0:D], cs["bl"][:, lp, :],
                                     qkv[:, 2, :], start=True, stop=True)
                t2 = act.tile([P, L, H, DH], bf16, tag="tbig")
                a_bcast = _fancy(a_t[:, 0, 0:1], [[H, L], [1, H], [0, DH]])
                vx_view = _fancy(vx[:, 0, 0:D], [[512, L], [DH, H], [1, DH]])
                nc.vector.tensor_mul(t2[:], a_bcast, vx_view)
                o_sb = act.tile([P, D], f32, tag="o_sb")
                nc.vector.reduce_sum(
                    out=o_sb[:],
                    in_=_fancy(t2[:, 0, 0, :], [[DH, H], [1, DH], [H * DH, L]]),
                    axis=AX.X)

                # ---- out-proj + residual + LN2 ----
                oT = transpose3(lambda j: o_sb[:, j * P:(j + 1) * P], 3, "oT",
                                src_f32=True)
                h2ps = ps_med.tile([P, D], f32, tag="med")
                for k in range(3):
                    nc.tensor.matmul(h2ps[:, :], oT[:, k, :], cs["wso"][:, k, :],
                                     start=(k == 0), stop=False)
                nc.tensor.matmul(h2ps[:, :], cs["ones1"][:, :], cs["bso"][:, :],
                                 start=False, stop=True)
                r2 = act.tile([P, D], bf16, tag="r2")
                nc.vector.tensor_add(r2[:], h2ps[:, :], y1[:])
                rstd, nmr = _layernorm_stats(nc, pools, r2[:])
                y2 = act.tile([P, D], bf16, tag="y2")
                nc.vector.tensor_scalar(out=y2[:], in0=r2[:],
                                        scalar1=rstd[:, 0:1], scalar2=nmr[:, 0:1],
                                        op0=AL.mult, op1=AL.add)
                nc.vector.tensor_mul(y2[:], y2[:], cs["n2_g"][:])
                nc.vector.tensor_add(y2[:], y2[:], cs["n2_b"][:])

                # ---- FFN (hidden stays feature-major: gelu output == lhsT chunks) ----
                y2T = transpose3(lambda j: y2[:, j * P:(j + 1) * P], 3, "y2T")
                ff1 = ps_big.tile([P, 12, P], f32, tag="big")
                for cchunk in range(12):
                    for k in range(3):
                        nc.tensor.matmul(ff1[:, cchunk, :], cs["w1"][:, k, cchunk * P:(cchunk + 1) * P],
                                         y2T[:, k, :], start=(k == 0), stop=False)
                    nc.tensor.matmul(ff1[:, cchunk, :],
                                     cs["b1row"][:, cchunk * P:(cchunk + 1) * P],
                                     cs["ones1"][:, :], start=False, stop=True)
                gl = act.tile([P, 12, P], bf16, tag="gl")
                nc.scalar.activation(out=gl[:], in_=ff1[:], func=AF.Gelu)
                ff2 = ps_med.tile([P, D], f32, tag="med")
                for k in range(12):
                    nc.tensor.matmul(ff2[:, :], gl[:, k, :], cs["w2"][:, k, :],
                                     start=(k == 0), stop=False)
                nc.tensor.matmul(ff2[:, :], cs["ones1"][:, :], cs["b2row"][:, :],
                                 start=False, stop=True)
                r3 = act.tile([P, D], bf16, tag="r3")
                nc.vector.tensor_add(r3[:], ff2[:, :], y2[:])
                rstd, nmr = _layernorm_stats(nc, pools, r3[:])
                y3 = act.tile([P, D], bf16, tag="y3")
                nc.vector.tensor_scalar(out=y3[:], in0=r3[:],
                                        scalar1=rstd[:, 0:1], scalar2=nmr[:, 0:1],
                                        op0=AL.mult, op1=AL.add)

                # ---- pool over l (accumulate [32,384] slices into macro psum) ----
                nc.tensor.matmul(poolps[32 * s:32 * (s + 1), :], cs["pool"][:, :],
                                 y3[:], start=True, stop=True,
                                 tile_position=(0, 32 * s))

            # ---- LN3 affine (folded post-pool) + gate + output ----
            pooled = io.tile([P, D], bf16, tag="pooled")
            nc.vector.tensor_mul(pooled[:], poolps[:, :], cs["n3_g"][:])
            nc.vector.tensor_add(pooled[:], pooled[:], cs["n3_b"][:])
            pT = transpose3(lambda j: pooled[:, j * P:(j + 1) * P], 3, "pT")
            gps = ps_med.tile([P, D], f32, tag="med")
            for k in range(3):
                nc.tensor.matmul(gps[:, :], pT[:, k, :], cs["wg"][:, k, :],
                                 start=(k == 0), stop=False)
            nc.tensor.matmul(gps[:, :], cs["ones1"][:, :], cs["bgrow"][:, :],
                             start=False, stop=True)
            gsig = io.tile([P, D], bf16, tag="gsig")
            nc.scalar.activation(out=gsig[:], in_=gps[:, :], func=AF.Sigmoid)
            outf = io.tile([P, D], f32, tag="outf")
            nc.vector.tensor_mul(outf[:], pooled[:], gsig[:])
            nc.sync.dma_start(out=out_d[m * P:(m + 1) * P, :], in_=outf[:])

    nc.finalize()   # Bacc: full compile pipeline (wait legalization etc.)
    return nc


_prog = None


def kernel(**inputs):
    global _prog
    inputs = {k: np.asarray(v, dtype=np.float32) for k, v in inputs.items()}
    consts = _host_consts(inputs)
    if _prog is None:
        _prog = build_program()
    x = inputs["x"]
    in_maps = []
    for c in range(NCORES):
        m = {"x": np.ascontiguousarray(x[c * BC:(c + 1) * BC])}
        m.update(consts)
        in_maps.append(m)
    res = run_bass_kernel_spmd(_prog, in_maps, core_ids=list(range(NCORES)))
    return np.concatenate([res.results[c]["out"] for c in range(NCORES)], axis=0)


if __name__ == "__main__":
    rng = np.random.default_rng(0)
    fake = {"x": rng.standard_normal((B, D), dtype=np.float32)}
    print("smoke build only")
    build_program()
    print("build OK")


# revision 11
# speedup vs baseline: 700.2440x; 3.1902x over previous
"""AttentionPooling Trainium2 kernel: 8-core data-parallel over batch.

Math notes (validated in check_math.py):
 - Cross-attention softmaxes over a single key position -> identity, so the
   whole first MHA collapses to  c = x @ Wc.T,  Wc = ca_w_out @ ca_wv,
   h1[b,l] = c[b] + (latents[l] + bc).
 - Self-attention over L=4 latents, H=8 heads, dh=48; 1/sqrt(dh) folded
   into Wq.  Softmax computed without max-subtraction (scores are tiny).
 - LN3 affine (g3, b3) folded after the mean-pool (pool is linear).

On-chip layout: rows = (b, l) pairs on partitions (l fastest), 128 rows per
subtile (32 batch); 4 subtiles per 128-batch macro tile; 32 macros per core.
GEMMs run on PE with activations transposed on-chip (PE transpose); norms and
softmax use per-partition fused DVE ops; transcendentals on ACT.
"""

from contextlib import ExitStack

import numpy as np
import ml_dtypes

import concourse.bass as bass
import concourse.bacc as bacc_mod
import concourse.tile as tile
from concourse import mybir
from concourse.bass_utils import run_bass_kernel_spmd

D, H, L, B, NCORES = 384, 8, 4, 32768, 8
DH = D // H                      # 48
BC = B // NCORES                 # 4096 rows per core
P = 128
NMAC = BC // P                   # 32 macro tiles per core
NSUB = 4                         # subtiles per macro (128 (b,l) rows each)
EPS = 1e-5

BF16 = ml_dtypes.bfloat16
f32 = mybir.dt.float32
bf16 = mybir.dt.bfloat16
AL = mybir.AluOpType
AF = mybir.ActivationFunctionType
AX = mybir.AxisListType


def _host_consts(inp):
    """All small parameters fused/packed on the host (numpy, f32 -> bf16)."""
    wq, wk, wv = np.split(inp["ca_w_in"], 3, axis=0)
    _, _, bv = np.split(inp["ca_b_in"], 3)
    Wc = inp["ca_w_out"] @ wv                              # [D, D]
    bc = inp["ca_w_out"] @ bv + inp["ca_b_out"]            # [D]
    latb = inp["latents"][0] + bc[None, :]                 # [L, D]

    WsaT = inp["sa_w_in"].T.copy()                         # [D, 3D]
    WsaT[:, :D] *= 1.0 / np.sqrt(DH)
    bqkv = inp["sa_b_in"].copy()
    bqkv[:D] *= 1.0 / np.sqrt(DH)

    def chunkT(wT, nk):  # [D_in, N] -> [128, nk, N] (k-chunks of 128 on partitions)
        n = wT.shape[1]
        return np.ascontiguousarray(wT.reshape(nk, P, n).transpose(1, 0, 2))

    c = {}
    c["wc"] = chunkT(Wc.T, 3)                              # [128, 3, 384]
    c["wsa"] = chunkT(WsaT, 3)                             # [128, 3, 1152]
    c["wso"] = chunkT(inp["sa_w_out"].T, 3)                # [128, 3, 384]
    c["w1"] = chunkT(inp["ffn_w1"].T, 3)                   # [128, 3, 1536]
    c["w2"] = chunkT(inp["ffn_w2"].T, 12)                  # [128, 12, 384]
    c["wg"] = chunkT(inp["gate_w"].T, 3)                   # [128, 3, 384]
    c["latb"] = latb                                       # [4, 384]

    # expansion / broadcast / pooling matrices
    pidx = np.arange(P)
    Eall = np.zeros((P, NSUB, P), np.float32)              # lhsT: [b128, s, p]
    for s in range(NSUB):
        Eall[32 * s + pidx // L, s, pidx] = 1.0
    c["emat"] = Eall
    oneL = np.zeros((L, P), np.float32)
    oneL[pidx % L, pidx] = 1.0
    c["onel"] = oneL                                       # [4, 128]
    Bl = np.zeros((P, L, P), np.float32)                   # lhsT: [p', l', p]
    for lp in range(L):
        Bl[(pidx // L) * L + lp, lp, pidx] = 1.0
    c["bl"] = Bl
    pm = np.zeros((P, 32), np.float32)
    pm[pidx, pidx // L] = 0.25
    c["pool"] = pm                                         # [128, 32]
    c["ident"] = np.eye(P, dtype=np.float32)

    # bias rows for PE bias-add matmuls (rank-1 tricks)
    c["ones1"] = np.ones((1, P), np.float32)
    c["bqkv"] = bqkv[None, :]                              # [1, 1152]
    c["bso"] = inp["sa_b_out"][None, :]
    c["b1row"] = inp["ffn_b1"][None, :]                    # [1, 1536]
    c["b2row"] = inp["ffn_b2"][None, :]
    c["bgrow"] = inp["gate_b"][None, :]

    # replicated per-feature vectors [128, 384]
    for nm in ("n1_g", "n1_b", "n2_g", "n2_b", "n3_g", "n3_b"):
        c[nm] = np.broadcast_to(inp[nm][None, :], (P, D)).copy()
    return {k: v.astype(BF16) for k, v in c.items()}


def _fancy(apbase, free_dims, extra_elem_offset=0):
    """Build an AP with custom free dims [[step,count],...] on top of a tile AP."""
    return bass.AP(
        tensor=apbase.tensor,
        offset=apbase.offset + extra_elem_offset,
        ap=[apbase.ap[0]] + [list(d) for d in free_dims],
    )


def _layernorm_stats(nc, pools, src_ap, nparts=P):
    """-> (rstd[128,1] f32, nmr[128,1] f32) for per-partition LN over free dim."""
    st = pools["stat"].tile([nparts, 6], f32, tag="bnst")
    nc.vector.bn_stats(out=st[:, :], in_=src_ap)
    mv = pools["stat"].tile([nparts, 2], f32, tag="bnmv")
    nc.vector.bn_aggr(out=mv[:, :], in_=st[:, :])
    rstd = pools["stat"].tile([nparts, 1], f32, tag="rstd")
    nc.scalar.activation(out=rstd[:, :], in_=mv[:, 1:2], func=AF.Sqrt,
                         bias=pools["eps"][:nparts, :], scale=1.0)
    nc.vector.reciprocal(out=rstd[:, :], in_=rstd[:, :])
    nmr = pools["stat"].tile([nparts, 1], f32, tag="nmr")
    nc.vector.tensor_scalar(out=nmr[:, :], in0=mv[:, 0:1], scalar1=rstd[:, 0:1],
                            scalar2=-1.0, op0=AL.mult, op1=AL.mult)
    return rstd, nmr


def build_program(repeat=1):
    nc = bacc_mod.Bacc("TRN2", target_bir_lowering=False, debug=False,
                       num_devices=NCORES)
    x_d = nc.declare_dram_parameter("x", [BC, D], f32, isOutput=False)
    consts_meta = {
        "wc": [P, 3, D], "wsa": [P, 3, 3 * D], "wso": [P, 3, D],
        "w1": [P, 3, 4 * D], "w2": [P, 12, D], "wg": [P, 3, D],
        "latb": [L, D], "emat": [P, NSUB, P], "onel": [L, P],
        "bl": [P, L, P], "pool": [P, 32], "ident": [P, P],
        "ones1": [1, P], "bqkv": [1, 3 * D], "bso": [1, D],
        "b1row": [1, 4 * D], "b2row": [1, D], "bgrow": [1, D],
        "n1_g": [P, D], "n1_b": [P, D], "n2_g": [P, D], "n2_b": [P, D],
        "n3_g": [P, D], "n3_b": [P, D],
    }
    cd = {k: nc.declare_dram_parameter(k, shp, bf16, isOutput=False)
          for k, shp in consts_meta.items()}
    out_d = nc.declare_dram_parameter("out", [BC, D], f32, isOutput=True)

    with tile.TileContext(nc) as tc, ExitStack() as ctx:
        consts = ctx.enter_context(tc.tile_pool(name="consts", bufs=1))
        io = ctx.enter_context(tc.tile_pool(name="io", bufs=3))
        act = ctx.enter_context(tc.tile_pool(name="act", bufs=3))
        stat = ctx.enter_context(tc.tile_pool(name="stat", bufs=4))
        ps_med = ctx.enter_context(tc.tile_pool(name="ps_med", bufs=3, space="PSUM"))
        ps_big = ctx.enter_context(tc.tile_pool(name="ps_big", bufs=1, space="PSUM"))
        ps_pool = ctx.enter_context(tc.tile_pool(name="ps_pool", bufs=1, space="PSUM"))

        cs = {}
        for k, shp in consts_meta.items():
            cs[k] = consts.tile(shp, bf16, name=f"c_{k}", tag=f"c_{k}")
            nc.sync.dma_start(out=cs[k][:], in_=cd[k][:])
        eps_t = consts.tile([P, 1], f32, tag="eps")
        nc.vector.memset(eps_t[:, :], EPS)
        pools = {"stat": stat, "eps": eps_t}

        identf = consts.tile([P, P], f32, tag="identf")
        # f32 identity built on-chip from the bf16 one (copy casts)
        nc.vector.tensor_copy(out=identf[:], in_=cs["ident"][:])

        rep_ctx = tc.For_i(0, repeat, 1) if repeat > 1 else None

        def transpose3(src_ap_fn, nchunks, dst_tag, src_f32=False):
            """PE-transpose nchunks [128,128] blocks of a row-major tile -> bf16 [128,nchunks,128]."""
            dst = act.tile([P, nchunks, P], bf16, tag=dst_tag)
            for j in range(nchunks):
                tp = ps_med.tile([P, P], f32 if src_f32 else bf16, tag="med", name="tp")
                nc.tensor.transpose(tp[:, :], src_ap_fn(j),
                                    identf[:] if src_f32 else cs["ident"][:])
                nc.any.tensor_copy(out=dst[:, j, :], in_=tp[:, :])
            return dst

        if rep_ctx is not None:
            ctx.enter_context(rep_ctx)
        for m in range(NMAC):
            xt = io.tile([P, D], f32, tag="xin")
            nc.sync.dma_start(out=xt[:], in_=x_d[m * P:(m + 1) * P, :])
            xT = transpose3(lambda j: xt[:, j * P:(j + 1) * P], 3, "xT", src_f32=True)

            # c = x @ Wc.T  (batch-major out [128b, 384])
            cps = ps_med.tile([P, D], f32, tag="med")
            for k in range(3):
                nc.tensor.matmul(cps[:, :], xT[:, k, :], cs["wc"][:, k, :],
                                 start=(k == 0), stop=(k == 2))
            c_sb = io.tile([P, D], bf16, tag="c_sb")
            nc.any.tensor_copy(out=c_sb[:], in_=cps[:, :])

            poolps = ps_pool.tile([P, D], f32, tag="poolacc")

            for s in range(NSUB):
                # ---- h1 = expand(c) + latb ; LN1 ----
                h1ps = ps_med.tile([P, D], f32, tag="med")
                nc.tensor.matmul(h1ps[:, :], cs["emat"][:, s, :], c_sb[:],
                                 start=True, stop=False)
                nc.tensor.matmul(h1ps[:, :], cs["onel"][:, :], cs["latb"][:, :],
                                 start=False, stop=True)
                rstd, nmr = _layernorm_stats(nc, pools, h1ps[:, :])
                y0 = act.tile([P, D], bf16, tag="y0")
                nc.vector.tensor_scalar(out=y0[:], in0=h1ps[:, :],
                                        scalar1=rstd[:, 0:1], scalar2=nmr[:, 0:1],
                                        op0=AL.mult, op1=AL.add)
                y1 = act.tile([P, D], bf16, tag="y1")
                nc.gpsimd.tensor_mul(y1[:], y0[:], cs["n1_g"][:])
                nc.any.tensor_add(y1[:], y1[:], cs["n1_b"][:])

                # ---- qkv GEMM ----
                y1T = transpose3(lambda j: y1[:, j * P:(j + 1) * P], 3, "y1T")
                qkvps = ps_big.tile([P, 4, 512], f32, tag="big")
                for part in range(3):          # q, k, v
                    for k in range(3):
                        nc.tensor.matmul(qkvps[:, part, 0:D], y1T[:, k, :],
                                         cs["wsa"][:, k, part * D:(part + 1) * D],
                                         start=(k == 0), stop=False)
                    nc.tensor.matmul(qkvps[:, part, 0:D], cs["ones1"][:, :],
                                     cs["bqkv"][:, part * D:(part + 1) * D],
                                     start=False, stop=True)
                qkv = act.tile([P, 3, D], bf16, tag="qkv")
                for part in range(3):
                    nc.any.tensor_copy(out=qkv[:, part, :], in_=qkvps[:, part, 0:D])

                # ---- scores: s[p, l', h] = sum_d q[p,h,d] * k[(b,l'),h,d] ----
                kx = ps_big.tile([P, L, 512], f32, tag="big")
                for lp in range(L):
                    nc.tensor.matmul(kx[:, lp, 0:D], cs["bl"][:, lp, :],
                                     qkv[:, 1, :], start=True, stop=True)
                t1 = act.tile([P, L, H, DH], bf16, tag="tbig")
                q_bcast = _fancy(qkv[:, 0, :], [[0, L], [DH, H], [1, DH]])
                kx_view = _fancy(kx[:, 0, 0:D], [[512, L], [DH, H], [1, DH]])
                nc.vector.tensor_mul(t1[:], q_bcast, kx_view)
                s_f = act.tile([P, L, H], f32, tag="s_f")
                nc.vector.reduce_sum(out=s_f[:], in_=t1[:], axis=AX.X)

                # softmax over l' (no max-subtract; scores are tiny)
                e_t = act.tile([P, L, H], f32, tag="e_t")
                nc.scalar.activation(out=e_t[:], in_=s_f[:], func=AF.Exp)
                z_t = act.tile([P, H], f32, tag="z_t")
                nc.vector.reduce_sum(out=z_t[:],
                                     in_=_fancy(e_t[:, 0, :], [[1, H], [H, L]]),
                                     axis=AX.X)
                nc.vector.reciprocal(out=z_t[:], in_=z_t[:])
                a_t = act.tile([P, L, H], f32, tag="a_t")
                nc.vector.tensor_mul(a_t[:], e_t[:],
                                     _fancy(z_t[:, :], [[0, L], [1, H]]))

                # ---- o = sum_l' a * v ----
                vx = ps_big.tile([P, L, 512], f32, tag="big")
                for lp in range(L):
                    nc.tensor.matmul(vx[:, lp, 0:D], cs["bl"][:, lp, :],
                                     qkv[:, 2, :], start=True, stop=True)
                t2 = act.tile([P, L, H, DH], bf16, tag="tbig")
                a_bcast = _fancy(a_t[:, 0, 0:1], [[H, L], [1, H], [0, DH]])
                vx_view = _fancy(vx[:, 0, 0:D], [[512, L], [DH, H], [1, DH]])
                nc.vector.tensor_mul(t2[:], a_bcast, vx_view)
                o_sb = act.tile([P, D], f32, tag="o_sb")
                nc.vector.reduce_sum(
                    out=o_sb[:],
                    in_=_fancy(t2[:, 0, 0, :], [[DH, H], [1, DH], [H * DH, L]]),
                    axis=AX.X)

                # ---- out-proj + residual + LN2 ----
                oT = transpose3(lambda j: o_sb[:, j * P:(j + 1) * P], 3, "oT",
                                src_f32=True)
                h2ps = ps_med.tile([P, D], f32, tag="med")
                for k in range(3):
                    nc.tensor.matmul(h2ps[:, :], oT[:, k, :], cs["wso"][:, k, :],
                                     start=(k == 0), stop=False)
                nc.tensor.matmul(h2ps[:, :], cs["ones1"][:, :], cs["bso"][:, :],
                                 start=False, stop=True)
                r2 = act.tile([P, D], bf16, tag="r2")
                nc.vector.tensor_add(r2[:], h2ps[:, :], y1[:])
                rstd, nmr = _layernorm_stats(nc, pools, r2[:])
                y2 = act.tile([P, D], bf16, tag="y2")
                nc.vector.tensor_scalar(out=y2[:], in0=r2[:],
                                        scalar1=rstd[:, 0:1], scalar2=nmr[:, 0:1],
                                        op0=AL.mult, op1=AL.add)
                nc.gpsimd.tensor_mul(y2[:], y2[:], cs["n2_g"][:])
                nc.any.tensor_add(y2[:], y2[:], cs["n2_b"][:])

                # ---- FFN (hidden stays feature-major: gelu output == lhsT chunks) ----
                y2T = transpose3(lambda j: y2[:, j * P:(j + 1) * P], 3, "y2T")
                ff1 = ps_big.tile([P, 12, P], f32, tag="big")
                for cchunk in range(12):
                    for k in range(3):
                        nc.tensor.matmul(ff1[:, cchunk, :], cs["w1"][:, k, cchunk * P:(cchunk + 1) * P],
                                         y2T[:, k, :], start=(k == 0), stop=False)
                    nc.tensor.matmul(ff1[:, cchunk, :],
                                     cs["b1row"][:, cchunk * P:(cchunk + 1) * P],
                                     cs["ones1"][:, :], start=False, stop=True)
                gl = act.tile([P, 12, P], bf16, tag="gl")
                nc.scalar.activation(out=gl[:], in_=ff1[:], func=AF.Gelu)
                ff2 = ps_med.tile([P, D], f32, tag="med")
                for k in range(12):
                    nc.tensor.matmul(ff2[:, :], gl[:, k, :], cs["w2"][:, k, :],
                                     start=(k == 0), stop=False)
                nc.tensor.matmul(ff2[:, :], cs["ones1"][:, :], cs["b2row"][:, :],
                                 start=False, stop=True)
                r3 = act.tile([P, D], bf16, tag="r3")
                nc.vector.tensor_add(r3[:], ff2[:, :], y2[:])
                rstd, nmr = _layernorm_stats(nc, pools, r3[:])
                y3 = act.tile([P, D], bf16, tag="y3")
                nc.vector.tensor_scalar(out=y3[:], in0=r3[:],
                                        scalar1=rstd[:, 0:1], scalar2=nmr[:, 0:1],
                                        op0=AL.mult, op1=AL.add)

                # ---- pool over l (accumulate [32,384] slices into macro psum) ----
                nc.tensor.matmul(poolps[32 * s:32 * (s + 1), :], cs["pool"][:, :],
                                 y3[:], start=True, stop=True,
                                 tile_position=(0, 32 * s))

            # ---- LN3 affine (folded post-pool) + gate + output ----
            pooled = io.tile([P, D], bf16, tag="pooled")
            nc.vector.tensor_mul(pooled[:], poolps[:, :], cs["n3_g"][:])
            nc.any.tensor_add(pooled[:], pooled[:], cs["n3_b"][:])
            pT = transpose3(lambda j: pooled[:, j * P:(j + 1) * P], 3, "pT")
            gps = ps_med.tile([P, D], f32, tag="med")
            for k in range(3):
                nc.tensor.matmul(gps[:, :], pT[:, k, :], cs["wg"][:, k, :],
                                 start=(k == 0), stop=False)
            nc.tensor.matmul(gps[:, :], cs["ones1"][:, :], cs["bgrow"][:, :],
                             start=False, stop=True)
            gsig = io.tile([P, D], bf16, tag="gsig")
            nc.scalar.activation(out=gsig[:], in_=gps[:, :], func=AF.Sigmoid)
            outf = io.tile([P, D], f32, tag="outf")
            nc.gpsimd.tensor_mul(outf[:], pooled[:], gsig[:])
            nc.sync.dma_start(out=out_d[m * P:(m + 1) * P, :], in_=outf[:])

    nc.finalize()   # Bacc: full compile pipeline (wait legalization etc.)
    return nc


_prog = None


def kernel(**inputs):
    global _prog
    inputs = {k: np.asarray(v, dtype=np.float32) for k, v in inputs.items()}
    consts = _host_consts(inputs)
    if _prog is None:
        _prog = build_program()
    x = inputs["x"]
    in_maps = []
    for c in range(NCORES):
        m = {"x": np.ascontiguousarray(x[c * BC:(c + 1) * BC])}
        m.update(consts)
        in_maps.append(m)
    res = run_bass_kernel_spmd(_prog, in_maps, core_ids=list(range(NCORES)))
    return np.concatenate([res.results[c]["out"] for c in range(NCORES)], axis=0)


if __name__ == "__main__":
    rng = np.random.default_rng(0)
    fake = {"x": rng.standard_normal((B, D), dtype=np.float32)}
    print("smoke build only")
    build_program()
    print("build OK")


# revision 13
# speedup vs baseline: 860.5019x; 1.2289x over previous
"""AttentionPooling Trainium2 kernel: 8-core data-parallel over batch.

Math notes (validated in check_math.py):
 - Cross-attention softmaxes over a single key position -> identity, so the
   whole first MHA collapses to  c = x @ Wc.T,  Wc = ca_w_out @ ca_wv,
   h1[b,l] = c[b] + (latents[l] + bc).
 - Self-attention over L=4 latents, H=8 heads, dh=48; 1/sqrt(dh) folded
   into Wq.  Softmax computed without max-subtraction (scores are tiny).
 - LN3 affine (g3, b3) folded after the mean-pool (pool is linear).

On-chip layout: rows = (b, l) pairs on partitions (l fastest), 128 rows per
subtile (32 batch); 4 subtiles per 128-batch macro tile; 32 macros per core.
GEMMs run on PE with activations transposed on-chip (PE transpose); norms and
softmax use per-partition fused DVE ops; transcendentals on ACT.
"""

from contextlib import ExitStack

import numpy as np
import ml_dtypes

import concourse.bass as bass
import concourse.bacc as bacc_mod
import concourse.tile as tile
from concourse import mybir
from concourse.bass_utils import run_bass_kernel_spmd

D, H, L, B, NCORES = 384, 8, 4, 32768, 8
DH = D // H                      # 48
BC = B // NCORES                 # 4096 rows per core
P = 128
NMAC = BC // P                   # 32 macro tiles per core
NSUB = 4                         # subtiles per macro (128 (b,l) rows each)
EPS = 1e-5

BF16 = ml_dtypes.bfloat16
f32 = mybir.dt.float32
bf16 = mybir.dt.bfloat16
AL = mybir.AluOpType
AF = mybir.ActivationFunctionType
AX = mybir.AxisListType


def _host_consts(inp):
    """All small parameters fused/packed on the host (numpy, f32 -> bf16)."""
    wq, wk, wv = np.split(inp["ca_w_in"], 3, axis=0)
    _, _, bv = np.split(inp["ca_b_in"], 3)
    Wc = inp["ca_w_out"] @ wv                              # [D, D]
    bc = inp["ca_w_out"] @ bv + inp["ca_b_out"]            # [D]
    latb = inp["latents"][0] + bc[None, :]                 # [L, D]

    WsaT = inp["sa_w_in"].T.copy()                         # [D, 3D]
    WsaT[:, :D] *= 1.0 / np.sqrt(DH)
    bqkv = inp["sa_b_in"].copy()
    bqkv[:D] *= 1.0 / np.sqrt(DH)

    def chunkT(wT, nk):  # [D_in, N] -> [128, nk, N] (k-chunks of 128 on partitions)
        n = wT.shape[1]
        return np.ascontiguousarray(wT.reshape(nk, P, n).transpose(1, 0, 2))

    c = {}
    c["wc"] = chunkT(Wc.T, 3)                              # [128, 3, 384]
    c["wsa"] = chunkT(WsaT, 3)                             # [128, 3, 1152]
    c["wso"] = chunkT(inp["sa_w_out"].T, 3)                # [128, 3, 384]
    c["w1"] = chunkT(inp["ffn_w1"].T, 3)                   # [128, 3, 1536]
    c["w2"] = chunkT(inp["ffn_w2"].T, 12)                  # [128, 12, 384]
    c["wg"] = chunkT(inp["gate_w"].T, 3)                   # [128, 3, 384]
    c["latb"] = latb                                       # [4, 384]

    # expansion / broadcast / pooling matrices
    pidx = np.arange(P)
    Eall = np.zeros((P, NSUB, P), np.float32)              # lhsT: [b128, s, p]
    for s in range(NSUB):
        Eall[32 * s + pidx // L, s, pidx] = 1.0
    c["emat"] = Eall
    oneL = np.zeros((L, P), np.float32)
    oneL[pidx % L, pidx] = 1.0
    c["onel"] = oneL                                       # [4, 128]
    Bl = np.zeros((P, L, P), np.float32)                   # lhsT: [p', l', p]
    for lp in range(L):
        Bl[(pidx // L) * L + lp, lp, pidx] = 1.0
    c["bl"] = Bl
    pm = np.zeros((P, 32), np.float32)
    pm[pidx, pidx // L] = 0.25
    c["pool"] = pm                                         # [128, 32]
    c["ident"] = np.eye(P, dtype=np.float32)

    # bias rows for PE bias-add matmuls (rank-1 tricks)
    c["ones1"] = np.ones((1, P), np.float32)
    c["bqkv"] = bqkv[None, :]                              # [1, 1152]
    c["bso"] = inp["sa_b_out"][None, :]
    c["b1row"] = inp["ffn_b1"][None, :]                    # [1, 1536]
    c["b2row"] = inp["ffn_b2"][None, :]
    c["bgrow"] = inp["gate_b"][None, :]

    # replicated per-feature vectors [128, 384]
    for nm in ("n1_g", "n1_b", "n2_g", "n2_b", "n3_g", "n3_b"):
        c[nm] = np.broadcast_to(inp[nm][None, :], (P, D)).copy()
    return {k: v.astype(BF16) for k, v in c.items()}


def _fancy(apbase, free_dims, extra_elem_offset=0):
    """Build an AP with custom free dims [[step,count],...] on top of a tile AP."""
    return bass.AP(
        tensor=apbase.tensor,
        offset=apbase.offset + extra_elem_offset,
        ap=[apbase.ap[0]] + [list(d) for d in free_dims],
    )


def _layernorm_stats(nc, pools, src_ap, nparts=P):
    """-> (rstd[128,1] f32, nmr[128,1] f32) for per-partition LN over free dim."""
    st = pools["stat"].tile([nparts, 6], f32, tag="bnst")
    nc.vector.bn_stats(out=st[:, :], in_=src_ap)
    mv = pools["stat"].tile([nparts, 2], f32, tag="bnmv")
    nc.vector.bn_aggr(out=mv[:, :], in_=st[:, :])
    rstd = pools["stat"].tile([nparts, 1], f32, tag="rstd")
    nc.scalar.activation(out=rstd[:, :], in_=mv[:, 1:2], func=AF.Sqrt,
                         bias=pools["eps"][:nparts, :], scale=1.0)
    nc.vector.reciprocal(out=rstd[:, :], in_=rstd[:, :])
    nmr = pools["stat"].tile([nparts, 1], f32, tag="nmr")
    nc.vector.tensor_scalar(out=nmr[:, :], in0=mv[:, 0:1], scalar1=rstd[:, 0:1],
                            scalar2=-1.0, op0=AL.mult, op1=AL.mult)
    return rstd, nmr


def build_program(repeat=1):
    nc = bacc_mod.Bacc("TRN2", target_bir_lowering=False, debug=False,
                       num_devices=NCORES)
    x_d = nc.declare_dram_parameter("x", [BC, D], f32, isOutput=False)
    consts_meta = {
        "wc": [P, 3, D], "wsa": [P, 3, 3 * D], "wso": [P, 3, D],
        "w1": [P, 3, 4 * D], "w2": [P, 12, D], "wg": [P, 3, D],
        "latb": [L, D], "emat": [P, NSUB, P], "onel": [L, P],
        "bl": [P, L, P], "pool": [P, 32], "ident": [P, P],
        "ones1": [1, P], "bqkv": [1, 3 * D], "bso": [1, D],
        "b1row": [1, 4 * D], "b2row": [1, D], "bgrow": [1, D],
        "n1_g": [P, D], "n1_b": [P, D], "n2_g": [P, D], "n2_b": [P, D],
        "n3_g": [P, D], "n3_b": [P, D],
    }
    cd = {k: nc.declare_dram_parameter(k, shp, bf16, isOutput=False)
          for k, shp in consts_meta.items()}
    out_d = nc.declare_dram_parameter("out", [BC, D], f32, isOutput=True)

    with tile.TileContext(nc) as tc, ExitStack() as ctx:
        consts = ctx.enter_context(tc.tile_pool(name="consts", bufs=1))
        io = ctx.enter_context(tc.tile_pool(name="io", bufs=3))
        act = ctx.enter_context(tc.tile_pool(name="act", bufs=3))
        stat = ctx.enter_context(tc.tile_pool(name="stat", bufs=4))
        ps_med = ctx.enter_context(tc.tile_pool(name="ps_med", bufs=3, space="PSUM"))
        ps_big = ctx.enter_context(tc.tile_pool(name="ps_big", bufs=1, space="PSUM"))
        ps_pool = ctx.enter_context(tc.tile_pool(name="ps_pool", bufs=1, space="PSUM"))

        cs = {}
        for k, shp in consts_meta.items():
            cs[k] = consts.tile(shp, bf16, name=f"c_{k}", tag=f"c_{k}")
            nc.sync.dma_start(out=cs[k][:], in_=cd[k][:])
        eps_t = consts.tile([P, 1], f32, tag="eps")
        nc.vector.memset(eps_t[:, :], EPS)
        pools = {"stat": stat, "eps": eps_t}

        identf = consts.tile([P, P], f32, tag="identf")
        # f32 identity built on-chip from the bf16 one (copy casts)
        nc.vector.tensor_copy(out=identf[:], in_=cs["ident"][:])

        rep_ctx = tc.For_i(0, repeat, 1) if repeat > 1 else None

        def transpose3(src_ap_fn, nchunks, dst_tag, src_f32=False):
            """PE-transpose nchunks [128,128] blocks of a row-major tile -> bf16 [128,nchunks,128]."""
            dst = act.tile([P, nchunks, P], bf16, tag=dst_tag)
            for j in range(nchunks):
                tp = ps_med.tile([P, P], f32 if src_f32 else bf16, tag="med", name="tp")
                nc.tensor.transpose(tp[:, :], src_ap_fn(j),
                                    identf[:] if src_f32 else cs["ident"][:])
                nc.any.tensor_copy(out=dst[:, j, :], in_=tp[:, :])
            return dst

        if rep_ctx is not None:
            ctx.enter_context(rep_ctx)
        for m in range(NMAC):
            xt = io.tile([P, D], f32, tag="xin")
            nc.sync.dma_start(out=xt[:], in_=x_d[m * P:(m + 1) * P, :])
            xT = transpose3(lambda j: xt[:, j * P:(j + 1) * P], 3, "xT", src_f32=True)

            # c = x @ Wc.T  (batch-major out [128b, 384])
            cps = ps_med.tile([P, D], f32, tag="med")
            for k in range(3):
                nc.tensor.matmul(cps[:, :], xT[:, k, :], cs["wc"][:, k, :],
                                 start=(k == 0), stop=(k == 2))
            c_sb = io.tile([P, D], bf16, tag="c_sb")
            nc.any.tensor_copy(out=c_sb[:], in_=cps[:, :])

            poolps = ps_pool.tile([P, D], f32, tag="poolacc")

            for s in range(NSUB):
                # ---- h1 = expand(c) + latb ; LN1 ----
                h1ps = ps_med.tile([P, D], f32, tag="med")
                nc.tensor.matmul(h1ps[:, :], cs["emat"][:, s, :], c_sb[:],
                                 start=True, stop=False)
                nc.tensor.matmul(h1ps[:, :], cs["onel"][:, :], cs["latb"][:, :],
                                 start=False, stop=True)
                rstd, nmr = _layernorm_stats(nc, pools, h1ps[:, :])
                y0 = act.tile([P, D], bf16, tag="y0")
                nc.vector.tensor_scalar(out=y0[:], in0=h1ps[:, :],
                                        scalar1=rstd[:, 0:1], scalar2=nmr[:, 0:1],
                                        op0=AL.mult, op1=AL.add)
                y1 = act.tile([P, D], bf16, tag="y1")
                nc.gpsimd.tensor_mul(y1[:], y0[:], cs["n1_g"][:])
                nc.any.tensor_add(y1[:], y1[:], cs["n1_b"][:])

                # ---- qkv GEMM ----
                y1T = transpose3(lambda j: y1[:, j * P:(j + 1) * P], 3, "y1T")
                qkvps = ps_big.tile([P, 4, 512], f32, tag="big")
                for part in range(3):          # q, k, v
                    for k in range(3):
                        nc.tensor.matmul(qkvps[:, part, 0:D], y1T[:, k, :],
                                         cs["wsa"][:, k, part * D:(part + 1) * D],
                                         start=(k == 0), stop=False)
                    nc.tensor.matmul(qkvps[:, part, 0:D], cs["ones1"][:, :],
                                     cs["bqkv"][:, part * D:(part + 1) * D],
                                     start=False, stop=True)
                qkv = act.tile([P, 3, D], bf16, tag="qkv")
                nc.any.tensor_copy(out=qkv[:, :, :], in_=qkvps[:, 0:3, 0:D])

                # ---- scores: s[p, l', h] = sum_d q[p,h,d] * k[(b,l'),h,d] ----
                kx = ps_big.tile([P, L, 512], f32, tag="big")
                for lp in range(L):
                    nc.tensor.matmul(kx[:, lp, 0:D], cs["bl"][:, lp, :],
                                     qkv[:, 1, :], start=True, stop=True)
                t1 = act.tile([P, L, H, DH], bf16, tag="tbig")
                q_bcast = _fancy(qkv[:, 0, :], [[0, L], [DH, H], [1, DH]])
                kx_view = _fancy(kx[:, 0, 0:D], [[512, L], [DH, H], [1, DH]])
                nc.any.tensor_mul(t1[:], q_bcast, kx_view)
                s_f = act.tile([P, L, H], f32, tag="s_f")
                nc.vector.reduce_sum(out=s_f[:], in_=t1[:], axis=AX.X)

                # softmax over l' (no max-subtract; scores are tiny)
                e_t = act.tile([P, L, H], f32, tag="e_t")
                nc.scalar.activation(out=e_t[:], in_=s_f[:], func=AF.Exp)
                z_t = act.tile([P, H], f32, tag="z_t")
                nc.vector.reduce_sum(out=z_t[:],
                                     in_=_fancy(e_t[:, 0, :], [[1, H], [H, L]]),
                                     axis=AX.X)
                nc.vector.reciprocal(out=z_t[:], in_=z_t[:])
                a_t = act.tile([P, L, H], f32, tag="a_t")
                nc.vector.tensor_mul(a_t[:], e_t[:],
                                     _fancy(z_t[:, :], [[0, L], [1, H]]))

                # ---- o = sum_l' a * v ----
                vx = ps_big.tile([P, L, 512], f32, tag="big")
                for lp in range(L):
                    nc.tensor.matmul(vx[:, lp, 0:D], cs["bl"][:, lp, :],
                                     qkv[:, 2, :], start=True, stop=True)
                t2 = act.tile([P, L, H, DH], bf16, tag="tbig")
                a_bcast = _fancy(a_t[:, 0, 0:1], [[H, L], [1, H], [0, DH]])
                vx_view = _fancy(vx[:, 0, 0:D], [[512, L], [DH, H], [1, DH]])
                nc.any.tensor_mul(t2[:], a_bcast, vx_view)
                o_sb = act.tile([P, D], f32, tag="o_sb")
                o_tmp = act.tile([P, 2, D], bf16, tag="o_tmp")
                nc.gpsimd.tensor_add(o_tmp[:, 0, :], t2[:, 0, :, :], t2[:, 1, :, :])
                nc.gpsimd.tensor_add(o_tmp[:, 1, :], t2[:, 2, :, :], t2[:, 3, :, :])
                nc.gpsimd.tensor_add(o_sb[:], o_tmp[:, 0, :], o_tmp[:, 1, :])

                # ---- out-proj + residual + LN2 ----
                oT = transpose3(lambda j: o_sb[:, j * P:(j + 1) * P], 3, "oT",
                                src_f32=True)
                h2ps = ps_med.tile([P, D], f32, tag="med")
                for k in range(3):
                    nc.tensor.matmul(h2ps[:, :], oT[:, k, :], cs["wso"][:, k, :],
                                     start=(k == 0), stop=False)
                nc.tensor.matmul(h2ps[:, :], cs["ones1"][:, :], cs["bso"][:, :],
                                 start=False, stop=True)
                r2 = act.tile([P, D], bf16, tag="r2")
                nc.vector.tensor_add(r2[:], h2ps[:, :], y1[:])
                rstd, nmr = _layernorm_stats(nc, pools, r2[:])
                y2 = act.tile([P, D], bf16, tag="y2")
                nc.vector.tensor_scalar(out=y2[:], in0=r2[:],
                                        scalar1=rstd[:, 0:1], scalar2=nmr[:, 0:1],
                                        op0=AL.mult, op1=AL.add)
                nc.gpsimd.tensor_mul(y2[:], y2[:], cs["n2_g"][:])
                nc.any.tensor_add(y2[:], y2[:], cs["n2_b"][:])

                # ---- FFN (hidden stays feature-major: gelu output == lhsT chunks) ----
                y2T = transpose3(lambda j: y2[:, j * P:(j + 1) * P], 3, "y2T")
                ff1 = ps_big.tile([P, 12, P], f32, tag="big")
                for cchunk in range(12):
                    for k in range(3):
                        nc.tensor.matmul(ff1[:, cchunk, :], cs["w1"][:, k, cchunk * P:(cchunk + 1) * P],
                                         y2T[:, k, :], start=(k == 0), stop=False)
                    nc.tensor.matmul(ff1[:, cchunk, :],
                                     cs["b1row"][:, cchunk * P:(cchunk + 1) * P],
                                     cs["ones1"][:, :], start=False, stop=True)
                gl = act.tile([P, 12, P], bf16, tag="gl")
                nc.scalar.activation(out=gl[:], in_=ff1[:], func=AF.Gelu)
                ff2 = ps_med.tile([P, D], f32, tag="med")
                for k in range(12):
                    nc.tensor.matmul(ff2[:, :], gl[:, k, :], cs["w2"][:, k, :],
                                     start=(k == 0), stop=False)
                nc.tensor.matmul(ff2[:, :], cs["ones1"][:, :], cs["b2row"][:, :],
                                 start=False, stop=True)
                r3 = act.tile([P, D], bf16, tag="r3")
                nc.vector.tensor_add(r3[:], ff2[:, :], y2[:])
                rstd, nmr = _layernorm_stats(nc, pools, r3[:])
                y3 = act.tile([P, D], bf16, tag="y3")
                nc.vector.tensor_scalar(out=y3[:], in0=r3[:],
                                        scalar1=rstd[:, 0:1], scalar2=nmr[:, 0:1],
                                        op0=AL.mult, op1=AL.add)

                # ---- pool over l (accumulate [32,384] slices into macro psum) ----
                nc.tensor.matmul(poolps[32 * s:32 * (s + 1), :], cs["pool"][:, :],
                                 y3[:], start=True, stop=True,
                                 tile_position=(0, 32 * s))

            # ---- LN3 affine (folded post-pool) + gate + output ----
            pooled = io.tile([P, D], bf16, tag="pooled")
            nc.vector.tensor_mul(pooled[:], poolps[:, :], cs["n3_g"][:])
            nc.any.tensor_add(pooled[:], pooled[:], cs["n3_b"][:])
            pT = transpose3(lambda j: pooled[:, j * P:(j + 1) * P], 3, "pT")
            gps = ps_med.tile([P, D], f32, tag="med")
            for k in range(3):
                nc.tensor.matmul(gps[:, :], pT[:, k, :], cs["wg"][:, k, :],
                                 start=(k == 0), stop=False)
            nc.tensor.matmul(gps[:, :], cs["ones1"][:, :], cs["bgrow"][:, :],
                             start=False, stop=True)
            gsig = io.tile([P, D], bf16, tag="gsig")
            nc.scalar.activation(out=gsig[:], in_=gps[:, :], func=AF.Sigmoid)
            outf = io.tile([P, D], f32, tag="outf")
            nc.gpsimd.tensor_mul(outf[:], pooled[:], gsig[:])
            nc.sync.dma_start(out=out_d[m * P:(m + 1) * P, :], in_=outf[:])

    nc.finalize()   # Bacc: full compile pipeline (wait legalization etc.)
    return nc


_prog = None


def kernel(**inputs):
    global _prog
    inputs = {k: np.asarray(v, dtype=np.float32) for k, v in inputs.items()}
    consts = _host_consts(inputs)
    if _prog is None:
        _prog = build_program()
    x = inputs["x"]
    in_maps = []
    for c in range(NCORES):
        m = {"x": np.ascontiguousarray(x[c * BC:(c + 1) * BC])}
        m.update(consts)
        in_maps.append(m)
    res = run_bass_kernel_spmd(_prog, in_maps, core_ids=list(range(NCORES)))
    return np.concatenate([res.results[c]["out"] for c in range(NCORES)], axis=0)


if __name__ == "__main__":
    rng = np.random.default_rng(0)
    fake = {"x": rng.standard_normal((B, D), dtype=np.float32)}
    print("smoke build only")
    build_program()
    print("build OK")
